# revision 34
# baseline (speedup 1.0000x reference)
"""Trainium2 Bass kernel for nn_AttentionBlock (B=2, S=4096, HID=256, 8 heads).

Sharding: 8 cores = 2 batches x 4 query-chunks of 1024 queries.
Each core redundantly computes K/V projections for its batch over the
mask-compacted key set, then attention for its 1024 queries over all 8
heads, then the output projection. Host gathers by concatenation.

v4 structure (host-layout + lean exp pipeline + software-pipelined PE):
- Host pre-compacts keys (mask nonzero indices), pre-transposes q/k/v to
  hid-major fp16, pre-casts weights to fp16. No device-side gathers or
  transposes; phase A is pure projection matmuls.
- Key-tile count nkc = ceil(nk_max/128) (128-granular, not 512).
- qT is pre-scaled by ASC = (1024/ln2)/sqrt(32) at projection eviction, so
  score PSUM holds t = ASC*x. Exp is one op per [128,1024] tile,
  alternating engines per key-tile for balance:
    ACT: LUT exp with scale=ln2/1024 (exact)
    DVE: tensor_scalar +B16C -> int16, bitcast fp16 (single-sample
         Schraudolph, geometrically centered, +-2.98% sawtooth that
         averages out over ~2k keys)
- PV matmuls vs mask-augmented V tiles ([32 v | mask | 31 zeros] per head)
  accumulate numerators and denominators together: 2 col-packed waves per
  key-tile; 3 PE waves per (g,kt) cycle total.
- PE stream is software-pipelined: scores(kt+1) issue before PV(kt) so the
  PE never head-of-line blocks on the exp engines.
- PSUM: 3x [128,1024]f32 score tiles (6 banks) + 2 wt accumulators.
  Phase A projections and tail broadcast/outproj borrow score-pool tiles.
- Tail per 512-query chunk: denominator rows DMA-packed, reciprocal via
  int32 magic + 1 Newton step, broadcast to 128 partitions with one K=2
  matmul, normalize multiply on DVE, fused output projection against
  zero-padded Wo rows with bias via K=1 ones matmul. The qc0 tail's
  matmuls are deferred into qc1's pipeline; the final tail runs its g0
  half early (during g1 compute) to shorten the serial epilogue.
"""

import numpy as np

import concourse.bacc as bacc
import concourse.bass as bass
from concourse import mybir
from concourse.tile import TileContext
from concourse.bass_utils import run_bass_kernel_spmd

F32 = mybir.dt.float32
F16 = mybir.dt.float16
I16 = mybir.dt.int16
I32 = mybir.dt.int32
AF = mybir.ActivationFunctionType
ALU = mybir.AluOpType

HID = 256
HEADS = 8
DH = 32
SK = 4096
SQ = 1024   # queries per core
SCALE = 1.0 / np.sqrt(32.0)
A16 = 1024.0 / np.log(2.0)          # Schraudolph slope (fp16 format)
ASC = float(A16 * SCALE)            # folded into qT at projection
EXPS = float(np.log(2.0) / 1024.0)  # ACT exp scale on t-space scores
B16 = 15360.0                       # Schraudolph offset (15*1024)
# single-sample Schraudolph: shift so the multiplicative sawtooth error
# (1+f)/2^f has zero ARITHMETIC mean over uniform f: E[(1+f)2^-f] = 1.0406
B16C = float(B16 - 1024.0 * np.log2(1.0406))
MAGIC = 0x7EF311C3                  # fp32 reciprocal magic
RS = 2048.0                         # reciprocal output scaling

_CACHE = {}


def _build_nc(nkc):
    """nkc = number of 128-key tiles after mask compaction."""
    nch = (nkc + 3) // 4           # 512-key projection chunks
    skp = nch * 512                # padded key columns in kT/vT inputs
    nc = bacc.Bacc("TRN2", target_bir_lowering=False, debug=False,
                   num_devices=8)

    q_d = nc.dram_tensor("qt_in", [HID, SQ], F16, kind="ExternalInput").ap()
    kv_d = nc.dram_tensor("kvt_in", [HID, 2 * skp], F16,
                          kind="ExternalInput").ap()
    wq_d = nc.dram_tensor("wq", [HID, HID], F16, kind="ExternalInput").ap()
    wk_d = nc.dram_tensor("wk", [HID, HID], F16, kind="ExternalInput").ap()
    wv_d = nc.dram_tensor("wv", [HID, HID], F16, kind="ExternalInput").ap()
    wo_d = nc.dram_tensor("wo_arr", [128, 1024], F16, kind="ExternalInput").ap()
    bqk_d = nc.dram_tensor("bqk", [128, 4], F32, kind="ExternalInput").ap()
    bo_d = nc.dram_tensor("bo2", [1, HID], F16, kind="ExternalInput").ap()
    sel_d = nc.dram_tensor("sel2", [2, 128], F16, kind="ExternalInput").ap()
    vm_d = nc.dram_tensor("vm8", [128, nkc * 8], F16,
                          kind="ExternalInput").ap()
    out_d = nc.dram_tensor("out", [SQ, HID], F32, kind="ExternalOutput").ap()

    from contextlib import ExitStack

    with TileContext(nc) as tc, ExitStack() as top:
        const = top.enter_context(tc.tile_pool(name="const", bufs=1))
        persist = top.enter_context(tc.tile_pool(name="persist", bufs=1))
        pt_pool = top.enter_context(tc.tile_pool(name="pt", bufs=4))
        wc_pool = top.enter_context(tc.tile_pool(name="wc", bufs=8))
        dn_pool = top.enter_context(tc.tile_pool(name="dn", bufs=16))
        rs_pool = top.enter_context(tc.tile_pool(name="rs", bufs=8))
        osb_pool = top.enter_context(tc.tile_pool(name="osb", bufs=4))

        st_pool = top.enter_context(tc.tile_pool(name="stp", bufs=3,
                                                 space="PSUM"))
        wt_persist = top.enter_context(tc.tile_pool(name="wtp", bufs=1,
                                                    space="PSUM"))

        # round-robin DMA issue across engines (all idle at startup)
        dma_engines = [nc.sync, nc.scalar, nc.gpsimd]
        dma_i = [0]

        def dma(dst, src):
            e = dma_engines[dma_i[0] % len(dma_engines)]
            dma_i[0] += 1
            e.dma_start(dst, src)

        # ------------- inputs in consumption order -------------
        # each weight [256,256] loads as one DMA into [128,512]
        wq_sb = []
        wk_sb = []
        wv_sb = []
        for nm, d_ap, lst in (("wk", wk_d, wk_sb), ("wq", wq_d, wq_sb),
                              ("wv", wv_d, wv_sb)):
            wb = const.tile([128, 512], F16, name=f"{nm}_h")
            dma(wb.rearrange("p (t c) -> p t c", t=2),
                d_ap.rearrange("(t p) c -> p t c", t=2))
            lst.extend([wb[:, 0:256], wb[:, 256:512]])
        bqk_sb = const.tile([128, 4], F32, name="bqk_sb")
        dma(bqk_sb, bqk_d)
        bq_sb = bqk_sb[:, 0:2]
        bk_sb = bqk_sb[:, 2:4]

        # kv chunks: one DMA each -> [128, 2048] = [k_t0|v_t0|k_t1|v_t1]
        kv_raw = [None] * nch
        k_raw = [[None] * nch for _ in range(2)]
        v_raw = [[None] * nch for _ in range(2)]

        def load_chunk(c):
            kv = const.tile([128, 2048], F16, name=f"kvraw_{c}")
            src = kv_d.rearrange("(t p) (s q) -> p t s q", t=2, s=2)
            for t in range(2):
                dma(kv[:, t * 1024:(t + 1) * 1024].rearrange(
                        "p (s c) -> p s c", s=2),
                    src[:, t, :, c * 512:(c + 1) * 512])
            kv_raw[c] = kv
            for t in range(2):
                k_raw[t][c] = kv[:, (2 * t) * 512:(2 * t) * 512 + 512]
                v_raw[t][c] = kv[:, (2 * t + 1) * 512:(2 * t + 1) * 512 + 512]

        load_chunk(0)
        q_all = const.tile([128, 2 * SQ], F16, name="qraw")
        dma(q_all.rearrange("p (t c) -> p t c", t=2),
            q_d.rearrange("(t p) c -> p t c", t=2))
        q_raw = [q_all[:, 0:SQ], q_all[:, SQ:2 * SQ]]
        if nch > 1:
            load_chunk(1)
        vm_sb = const.tile([128, nkc * 8], F16, name="vm_sb")
        dma(vm_sb, vm_d)
        wo_sb = const.tile([128, 1024], F16, name="wo_sb")
        dma(wo_sb, wo_d)
        bo_sb = const.tile([1, HID], F16, name="bo_sb")
        dma(bo_sb, bo_d)
        sel2 = const.tile([128, 128], F16, name="sel2")
        dma(sel2[32:34, :], sel_d)
        for c in range(2, nch):
            load_chunk(c)

        ones_hf = const.tile([1, 128], F16, name="ones_hf")
        nc.vector.memset(ones_hf, 1.0)
        # preload the exp activation table while DMAs are in flight
        dumm = const.tile([1, 8], F32, name="dumm")
        nc.vector.memset(dumm, 0.0)
        dummo = const.tile([1, 8], F16, name="dummo")
        nc.scalar.activation(dummo, dumm, AF.Exp, scale=1.0)

        # ---------------- persistent buffers ----------------
        qT_sb = [persist.tile([128, SQ], F16, name=f"qT_sb{g}")
                 for g in range(2)]
        kT_ch = [[persist.tile([128, 512], F16, name=f"kT{g}_{c}")
                  for c in range(nch)] for g in range(2)]
        # augmented V: per head 33 cols = [32 v | mask]; PV matmuls are
        # M=33, so wt rows 33-63 / 97-127 are never written -- they are
        # zeroed once below and stay zero (nothing else touches them)
        vaug_all = persist.tile([128, nkc * 264], F16, name="vaug")
        vaug = [vaug_all[:, s * 264:(s + 1) * 264] for s in range(nkc)]
        wtn_all = [persist.tile([128, 512], F16, name=f"wtn{i}")
                   for i in range(4)]
        vdst = vaug_all.rearrange("p (s h e) -> p s h e", h=8, e=33)
        nc.vector.tensor_copy(
            vdst[:, :, :, 32:33],
            vm_sb.rearrange("p (s h e) -> p s h e", h=8, e=1))
        # persistent PV accumulator banks (write-after-read tracked by Tile)
        wt_ps = [wt_persist.tile([128, 512], F32, name=f"wtps{jj}")
                 for jj in range(2)]
        for jj in range(2):
            nc.vector.memset(wt_ps[jj], 0.0)

        # ---------------- phase A helpers ----------------
        def emit_k_chunk(c):
            for g in range(2):
                ps = st_pool.tile([128, 1024], F32, tag="st",
                                  name="kps")[:, 0:512]
                for t in range(2):
                    nc.tensor.matmul(
                        ps, wk_sb[t][:, g * 128:(g + 1) * 128], k_raw[t][c],
                        start=(t == 0), stop=(t == 1))
                nc.scalar.activation(kT_ch[g][c], ps, AF.Identity,
                                     bias=bk_sb[:, g:g + 1], scale=1.0)

        def emit_v_tile(s):
            c, r = divmod(s, 4)
            ps = st_pool.tile([128, 1024], F32, tag="st",
                              name="vps")[:, 0:256]
            for t in range(2):
                nc.tensor.matmul(
                    ps, v_raw[t][c][:, r * 128:(r + 1) * 128], wv_sb[t],
                    start=(t == 0), stop=(t == 1))
            dst = vaug[s].rearrange("p (h e) -> p h e", e=33)[:, :, 0:DH]
            src = ps.rearrange("p (h e) -> p h e", e=DH)
            nc.vector.tensor_copy(dst, src)

        def emit_q():
            for g in range(2):
                for cq in range(2):
                    ps = st_pool.tile([128, 1024], F32, tag="st",
                                      name="qps")[:, 0:512]
                    for t in range(2):
                        nc.tensor.matmul(
                            ps, wq_sb[t][:, g * 128:(g + 1) * 128],
                            q_raw[t][:, cq * 512:(cq + 1) * 512],
                            start=(t == 0), stop=(t == 1))
                    nc.scalar.activation(
                        qT_sb[g][:, cq * 512:(cq + 1) * 512], ps,
                        AF.Identity, bias=bq_sb[:, g:g + 1], scale=ASC)

        # chunk 0 + queries up front; chunks 1.. interleave into qc0/g0
        emit_k_chunk(0)
        emit_q()
        for s in range(min(4, nkc)):
            emit_v_tile(s)

        # ---------------- attention building blocks ----------------
        def emit_scores(qc, g, kt):
            c, r = divmod(kt, 4)
            sts = [st_pool.tile([128, 1024], F32, tag="st", name="st")
                   for _ in range(2)]
            for jj in range(2):
                for j2 in range(2):
                    j = 2 * jj + j2
                    nc.tensor.matmul(
                        sts[jj][:, j2 * 512:(j2 + 1) * 512],
                        kT_ch[g][c][32 * j:32 * j + 32,
                                    r * 128:r * 128 + 128],
                        qT_sb[g][32 * j:32 * j + 32,
                                 qc * 512:(qc + 1) * 512],
                        start=True, stop=True,
                        tile_position=(32 * j, 0))
            return sts

        def emit_exp(kt, sts):
            # jj0 (first-allocated PSUM tile) always on the faster ACT so
            # the next key-tile's score matmuls unblock sooner
            pts = []
            for jj in range(2):
                pt = pt_pool.tile([128, 1024], F16, tag="pt", name="pt")
                if jj == 1:
                    nc.vector.tensor_scalar(
                        pt.bitcast(I16), sts[jj], 1.0, B16C,
                        op0=ALU.mult, op1=ALU.add)
                else:
                    nc.scalar.activation(pt, sts[jj], AF.Exp, scale=EXPS)
                pts.append(pt)
            return pts

        def emit_pv(g, kt, wts, pts):
            for jj in range(2):
                for j2 in range(2):
                    h = 4 * g + 2 * jj + j2
                    nc.tensor.matmul(
                        wts[jj][64 * j2:64 * j2 + 33, :],
                        vaug[kt][:, 33 * h:33 * h + 33],
                        pts[jj][:, j2 * 512:(j2 + 1) * 512],
                        start=(kt == 0), stop=(kt == nkc - 1),
                        tile_position=(0, 64 * j2),
                        skip_group_check=True)

        def emit_wcop(g, wts, wcops, dpacks_g, fast=False):
            for jj in range(2):
                wcop = wc_pool.tile([128, 512], F32, tag="wcop",
                                    name="wcop")
                if fast and jj == 1:
                    # parallel eviction on DVE to shorten the epilogue
                    nc.vector.tensor_copy(wcop, wts[jj])
                else:
                    nc.scalar.activation(wcop, wts[jj], AF.Copy)
                dp = dpacks_g[jj]
                (nc.sync if jj == 0 else nc.scalar).dma_start(
                    dp[0:1, :], wcop[32:33, :])
                (nc.gpsimd if jj == 0 else nc.sync).dma_start(
                    dp[1:2, :], wcop[96:97, :])
                wcops.append(wcop)

        def emit_recip(eng, dpack):
            """reciprocal of dpack [2,512] -> 2048/d fp16; 4 serial ops."""
            r0i = dn_pool.tile([2, 512], I32, tag="dp", name="r0i")
            eng.tensor_scalar(r0i, dpack.bitcast(I32), -1, MAGIC,
                              op0=ALU.mult, op1=ALU.add)
            r0 = r0i.bitcast(F32)
            t1 = dn_pool.tile([2, 512], F32, tag="dp", name="t1")
            eng.tensor_tensor(t1, dpack, r0, op=ALU.mult)
            t1b = dn_pool.tile([2, 512], F32, tag="dp", name="t1b")
            eng.tensor_scalar(t1b, t1, -RS, 2.0 * RS,
                              op0=ALU.mult, op1=ALU.add)
            r2h = dn_pool.tile([2, 512], F16, tag="dp", name="r2h")
            eng.tensor_tensor(r2h, r0, t1b, op=ALU.mult)
            return r2h

        def emit_norm(p, r2h, wcop):
            """broadcast reciprocal rows + normalize weight copy p."""
            rsp = rs_pool.tile([34, 512], F16, tag="rsp", name="rsp")
            nc.sync.dma_start(rsp[32:33, :], r2h[0:1, :])
            nc.scalar.dma_start(rsp[33:34, :], r2h[1:2, :])
            bc = st_pool.tile([128, 1024], F32, tag="st", name="bc")[:, 0:512]
            nc.tensor.matmul(bc, sel2[32:34, :], rsp[32:34, :],
                             start=True, stop=True, tile_position=(32, 0))
            nc.vector.tensor_tensor(wtn_all[p], wcop, bc, op=ALU.mult)

        def emit_outproj(qc, ms=(0, 1, 2, 3)):
            for m in ms:
                ops = st_pool.tile([128, 1024], F32, tag="st",
                                   name="ops")[:, 0:256]
                for p in range(4):
                    nc.tensor.matmul(
                        ops, wtn_all[p][:, m * 128:(m + 1) * 128],
                        wo_sb[:, p * 256:(p + 1) * 256],
                        start=(p == 0), stop=False, skip_group_check=True)
                nc.tensor.matmul(ops, ones_hf[0:1, :], bo_sb,
                                 start=False, stop=True,
                                 skip_group_check=True)
                ob = osb_pool.tile([128, 256], F32, tag="ob", name="ob")
                if m % 2 == 0:
                    nc.scalar.activation(ob, ops, AF.Copy)
                else:
                    nc.vector.tensor_copy(ob, ops)
                (nc.sync if m % 2 == 0 else nc.gpsimd).dma_start(
                    out_d[qc * 512 + m * 128:qc * 512 + (m + 1) * 128, :],
                    ob)

        # ---------------- attention main loop ----------------
        tail_q = []         # deferred tail pieces, drained one per key-tile
        last = SQ // 512 - 1
        for qc in range(SQ // 512):
            wcops = []
            dpacks = [[dn_pool.tile([2, 512], F32, tag="dp", name="dpack")
                       for _ in range(2)] for _ in range(2)]
            for g in range(2):
                wts = wt_ps
                prev = None
                for kt in range(nkc):
                    sts = emit_scores(qc, g, kt)
                    pts = emit_exp(kt, sts)
                    if prev is not None:
                        emit_pv(g, kt - 1, wts, prev)
                    prev = pts

                    # interleave remaining phase-A work into qc0/g0
                    if qc == 0 and g == 0 and kt % 4 == 2:
                        cc = kt // 4 + 1
                        if cc < nch:
                            emit_k_chunk(cc)
                            for s2 in range(cc * 4, min((cc + 1) * 4, nkc)):
                                emit_v_tile(s2)
                    # drain deferred tail pieces at odd key-tiles, late
                    # enough that the reciprocal chains have completed
                    if tail_q and kt % 2 == 1 and (g == 1 or kt >= 7):
                        tail_q.pop(0)()
                emit_pv(g, nkc - 1, wts, prev)
                emit_wcop(g, wts, wcops, dpacks[g],
                          fast=(qc == last and g == 1))

                if qc == last and g == 0:
                    # final tail's g0 half: reciprocal now (DVE+GS, no PE),
                    # normalize matmuls deferred into the g1 pipeline
                    r2h_g0 = [emit_recip(nc.vector, dpacks[0][0]),
                              emit_recip(nc.gpsimd, dpacks[0][1])]
                    for jj in range(2):
                        def fin_g0(jj=jj):
                            emit_norm(jj, r2h_g0[jj], wcops[jj])
                        tail_q.append(fin_g0)

            if qc < last:
                def make_tail(qc0, wcops0, dpacks0):
                    r2h_g = [[emit_recip(nc.gpsimd, dpacks0[g2][jj])
                              for jj in range(2)] for g2 in range(2)]
                    for g2 in range(2):
                        for jj in range(2):
                            def np_part(g2=g2, jj=jj):
                                p = 2 * g2 + jj
                                emit_norm(p, r2h_g[g2][jj], wcops0[p])
                            tail_q.append(np_part)

                    def op_part(ms):
                        def run():
                            emit_outproj(qc0, ms)
                        return run
                    tail_q.append(op_part((0, 1)))
                    tail_q.append(op_part((2, 3)))
                make_tail(qc, wcops, dpacks)

        # final epilogue: g1 half of the last tail + output projection
        for t in tail_q:
            t()
        r2h_b = [emit_recip(nc.vector, dpacks[1][0]),
                 emit_recip(nc.gpsimd, dpacks[1][1])]
        for jj in range(2):
            emit_norm(2 + jj, r2h_b[jj], wcops[2 + jj])
        emit_outproj(last, (0, 1, 2, 3))

    nc.finalize()
    return nc


def _get_nc(nkc):
    key = ("nc", nkc)
    if key not in _CACHE:
        _CACHE[key] = _build_nc(nkc)
    return _CACHE[key]


def kernel(query, key, value, mask, Wq, bq, Wk, bk, Wv, bv, Wo, bo,
           _trace=False):
    query = np.asarray(query, np.float32)
    key = np.asarray(key, np.float32)
    value = np.asarray(value, np.float32)
    mask = np.asarray(mask, np.int32)
    Wq = np.asarray(Wq, np.float32)
    Wk = np.asarray(Wk, np.float32)
    Wv = np.asarray(Wv, np.float32)
    Wo = np.asarray(Wo, np.float32)
    bq = np.asarray(bq, np.float32)
    bk = np.asarray(bk, np.float32)
    bv = np.asarray(bv, np.float32)
    bo = np.asarray(bo, np.float32)

    # mask compaction: indices of surviving keys per batch
    idxs = [np.nonzero(mask[b, 0])[0].astype(np.int32) for b in range(2)]
    nk_max = max(max(len(ix) for ix in idxs), 1)
    nkc = max((nk_max + 127) // 128, 4)
    nch = (nkc + 3) // 4
    skp = nch * 512

    nc = _get_nc(nkc)

    wo_arr = np.zeros((128, 4, 256), np.float32)
    for p in range(4):
        wo_arr[0:32, p] = Wo[64 * p:64 * p + 32]
        wo_arr[64:96, p] = Wo[64 * p + 32:64 * p + 64]
    wo_arr = np.ascontiguousarray(
        wo_arr.reshape(128, 1024).astype(np.float16))
    bqk = np.concatenate([bq.reshape(2, 128).T * ASC,
                          bk.reshape(2, 128).T], axis=1)
    bqk = np.ascontiguousarray(bqk).astype(np.float32)
    bo2 = np.ascontiguousarray(
        (bv @ Wo + bo).reshape(1, 256)).astype(np.float16)
    sel2 = np.zeros((2, 128), np.float16)
    sel2[0, 0:32] = 1.0 / RS
    sel2[1, 64:96] = 1.0 / RS
    wq16 = np.ascontiguousarray(Wq.astype(np.float16))
    wk16 = np.ascontiguousarray(Wk.astype(np.float16))
    wv16 = np.ascontiguousarray(Wv.astype(np.float16))

    # per-batch compacted, hid-major [k | v] + query transposes
    kvT_b = []
    vm_b = []
    for b in range(2):
        ix = idxs[b]
        nk = len(ix)
        kv = np.zeros((2 * skp, HID), np.float16)
        kv[:nk] = key[b][ix].astype(np.float16)
        kv[skp:skp + nk] = value[b][ix].astype(np.float16)
        kvT_b.append(np.ascontiguousarray(kv.T))
        mrow = (np.arange(nkc * 128) < nk).astype(np.float16)
        vm = np.repeat(mrow.reshape(nkc, 128, 1), 8, axis=2)  # [nkc,128,8]
        vm_b.append(np.ascontiguousarray(
            vm.transpose(1, 0, 2).reshape(128, nkc * 8)))

    in_maps = []
    for cidx in range(8):
        b, qi = divmod(cidx, 4)
        in_maps.append({
            "qt_in": np.ascontiguousarray(
                query[b, qi * SQ:(qi + 1) * SQ].astype(np.float16).T),
            "kvt_in": kvT_b[b],
            "wq": wq16, "wk": wk16, "wv": wv16, "wo_arr": wo_arr,
            "bqk": bqk, "bo2": bo2, "sel2": sel2,
            "vm8": vm_b[b],
        })

    res = run_bass_kernel_spmd(nc, in_maps, core_ids=list(range(8)),
                               trace=_trace)
    if _trace:
        _CACHE["last_result"] = res

    out = np.empty((2, 4096, HID), np.float32)
    for cidx in range(8):
        b, qi = divmod(cidx, 4)
        out[b, qi * SQ:(qi + 1) * SQ] = res.results[cidx]["out"]
    return out
